# revision 5
# baseline (speedup 1.0000x reference)
"""Trainium2 Bass kernel for nn_CNNModel_82222853915196.

Model (per utterance x: (64, 512)):
  multiscale patch features (h in {8,16,32,64}) -> feats (8192,)
  out[t, :] = Wfc @ concat([x[:, t], feats]) + bfc

Factorization: feats is broadcast over t, so
  out = x.T @ Wfc1.T  +  broadcast(C),  C = Wfc2 @ feats + cconst.

Structure (v4 — DMA-chase schedule over three queues):
 * Three parallel DMA queues (sync/scalar HWDGE + gpsimd SWDGE) with FAT
   descriptors only: HWDGE spreads a DMA over (largest divisor of its
   descriptor count <= 16) engines, and every DMA boundary costs ~1us of
   queue overhead, so descriptor counts are multiples of 16 and tensors
   are merged per-partition: xwpk = [x | wpack], wfc2m = [wfc2-h64 fp16 |
   wfc2-h32 fp16 | wfc2-lo fp8 bits packed as fp16] (device reads the lo
   region through an fp8 bitcast AP).
 * Landing order x/wpack -> w64wp -> wfc2 in 6 column chunks whose queue
   assignment makes completion order match the C-matmul program order
   (h64, h32, lo) so the in-order tensor engine chases the stream.
 * fp8 DoubleRow (both operands fp8, K=256/instruction) halves the
   C-matmul count for the h8/h16 blocks (16 kt-pair MMs, feats quantized
   fp8 with x4/:4 scale balancing) and the frames term (16 MMs over
   paired x rows, x/Wfc1 fp8 at 1/8 / x8 scales; frames is ~1.4% of
   output variance).  DR stationaries need per-plane free size in
   {32, 64, 128}, so the 4 real utterance columns are zero-padded to 32.
 * C accumulates in one [32, 400] PSUM bank (rows 0-3 real), seeded by a
   zero DR matmul so mixed DR/fp16 accumulation is defined.
 * Finalize: C+cconst -> 4 PE transposes -> 16 in-place column adds on
   outstage (scalar activation-bias / DVE / Pool tensor-tensor), output
   as 6 DMAs (3 queues x 2 column halves), each half fired as soon as
   its adds are done.

Sharding: pure data parallel - 32 utterances -> 8 cores x 4. Weights
replicated; no cross-core communication.
"""

import os
import sys
from contextlib import ExitStack

import numpy as np

for _p in ("/opt/trn_rl_repo", "/root/.axon_site/_ro/trn_rl_repo"):
    if os.path.isdir(_p) and _p not in sys.path:
        sys.path.insert(0, _p)

import concourse.bass as bass
import concourse.tile as tile
from concourse import bacc, mybir
from concourse.bass_utils import run_bass_kernel_spmd

NCORES = 8
NUTT = 4                 # utterances per core
T = 512
F = 64
OUT = 400
W = NUTT * T             # 2048, free width of the x tile
FP32 = mybir.dt.float32
FP16 = mybir.dt.float16
FP8 = mybir.dt.float8e4
NPF16 = np.float16
DRMODE = mybir.MatmulPerfMode.DoubleRow

# xwpk column offsets (fp16 [128, 6336]): x [128, 2048] then wpack:
#   w8jp [128,128] | w16jp [128,512] | w32jp [128,2048] | (unused 400)
#   | cconst4 [4,400] | eye32 [32,32] | S8 [32,512] | S16 [64,256]
WP_W8, WP_W16, WP_W32 = 0, 128, 640
WP_FC1, WP_CC, WP_EYE = 2688, 3088, 3488
WP_S8, WP_S16 = 3520, 4032
WP_COLS = 4288
XW_COLS = W + WP_COLS

# wfc2m (fp16 [128, 19200]): w64h [0:6400] | w32h [6400:12800] |
#   wlo fp8 bits as fp16 [12800:19200]
WM_H64, WM_H32, WM_LO = 0, 6400, 12800
WM_COLS = 19200

# xtra (fp8 [32, 5120]): xfp8 [32, 4u*(2,512)] | wfc1p [32, 4ot*(2,128)]
XT_X, XT_FC = 0, 4096
XT_COLS = 5120

# fp8 scale balancing (folded so products are exact):
#   wlo stored x4, feats stored /4 (via S8/S16 = eye/4)
#   wfc1p stored x8, xfp8 stored /8
WLO_SCALE = 4.0
FC1_SCALE = 8.0


# ---------------------------------------------------------------------------
# host-side weight preparation
# ---------------------------------------------------------------------------

def _build_devindex():
    """devindex[kt, fp] = reference flat feature index m in [0, 8192)."""
    devindex = np.full((64, 128), -1, dtype=np.int64)
    # h=8: PSUM (q=k*4+o, u*64+p): kt = p//4, fp = (p%4)*32 + q
    for k in range(8):
        for p in range(64):
            for o in range(4):
                devindex[p // 4, (p % 4) * 32 + k * 4 + o] = (k * 64 + p) * 4 + o
    # h=16: (q=k*16+o, u*32+p): kt = 16 + p//2, fp = (p%2)*64 + q
    for k in range(4):
        for p in range(32):
            for o in range(16):
                devindex[16 + p // 2, (p % 2) * 64 + k * 16 + o] = \
                    2048 + (k * 32 + p) * 16 + o
    # h=32: (q=k*64+o, u*16+p): kt = 32 + p, fp = q  (partition-preserving)
    for k in range(2):
        for p in range(16):
            for o in range(64):
                devindex[32 + p, k * 64 + o] = 4096 + (k * 16 + p) * 64 + o
    # h=64 via PE transpose: kt = 48 + g*8 + p (g = o//128), fp = o%128
    for p in range(8):
        for o in range(256):
            devindex[48 + (o // 128) * 8 + p, o % 128] = 6144 + p * 256 + o
    assert devindex.min() >= 0
    return devindex


def _masked_paired(Wh, nk, h, no):
    """wp[r or 64+r, m*nk*no + k*no + o] = Wh[k, o, (r-k)*h + (2m or 2m+1)]."""
    w = np.zeros((64, h, nk * no), dtype=np.float32)
    for k in range(nk):
        for i in range(h):
            w[k + i, :, k * no:(k + 1) * no] = Wh[k].reshape(no, h, h)[:, i, :].T
    wp = np.zeros((128, (h // 2) * nk * no), dtype=np.float32)
    q = nk * no
    for m in range(h // 2):
        wp[0:64, m * q:(m + 1) * q] = w[:, 2 * m, :]
        wp[64:128, m * q:(m + 1) * q] = w[:, 2 * m + 1, :]
    return wp


def host_prep(W8, b8, W16, b16, W32, b32, W64, b64, Wfc, bfc):
    f32 = np.float32
    W8 = np.asarray(W8, f32); W16 = np.asarray(W16, f32)
    W32 = np.asarray(W32, f32); W64 = np.asarray(W64, f32)
    Wfc = np.asarray(Wfc, f32)
    b8 = np.asarray(b8, f32); b16 = np.asarray(b16, f32)
    b32 = np.asarray(b32, f32); b64 = np.asarray(b64, f32)
    bfc = np.asarray(bfc, f32)
    import ml_dtypes
    np8 = ml_dtypes.float8_e4m3

    wpk = np.zeros((128, WP_COLS), dtype=f32)
    wpk[:, WP_W8:WP_W8 + 128] = _masked_paired(W8, 8, 8, 4)
    wpk[:, WP_W16:WP_W16 + 512] = _masked_paired(W16, 4, 16, 16)
    wpk[:, WP_W32:WP_W32 + 2048] = _masked_paired(W32, 2, 32, 64)
    wpk[0:32, WP_EYE:WP_EYE + 32] = np.eye(32, dtype=f32)
    # partition-expansion stationaries carry the 1/WLO_SCALE that balances
    # the x WLO_SCALE on the fp8 wlo values (both exact powers of two).
    for pl in range(4):
        blk = wpk[0:32, WP_S8 + pl * 128:WP_S8 + (pl + 1) * 128]
        blk[:, pl * 32:(pl + 1) * 32] = np.eye(32, dtype=f32) / WLO_SCALE
    for pl in range(2):
        blk = wpk[0:64, WP_S16 + pl * 128:WP_S16 + (pl + 1) * 128]
        blk[:, pl * 64:(pl + 1) * 64] = np.eye(64, dtype=f32) / WLO_SCALE

    # w64wp[i, m*256+o] = W64[o, i*64+2m]; row 64+i holds j=2m+1
    w64 = W64.reshape(256, 64, 64)              # [o, i, j]
    w64wp = np.zeros((128, 32 * 256), dtype=f32)
    for m in range(32):
        w64wp[0:64, m * 256:(m + 1) * 256] = w64[:, :, 2 * m].T
        w64wp[64:128, m * 256:(m + 1) * 256] = w64[:, :, 2 * m + 1].T

    devindex = _build_devindex()
    Wfc2 = Wfc[:, 64:]
    perm = Wfc2[:, devindex.reshape(-1)].T      # [8192, 400], kt-major rows
    wfc2f = np.ascontiguousarray(
        perm.reshape(64, 128, OUT).transpose(1, 0, 2).reshape(128, 64 * OUT))

    fb = np.zeros(8192, dtype=np.float64)
    fb[0:2048] = np.broadcast_to(b8[:, None, :], (8, 64, 4)).reshape(-1)
    fb[2048:4096] = np.broadcast_to(b16[:, None, :], (4, 32, 16)).reshape(-1)
    fb[4096:6144] = np.broadcast_to(b32[:, None, :], (2, 16, 64)).reshape(-1)
    fb[6144:8192] = np.broadcast_to(b64[None, :], (8, 256)).reshape(-1)
    cconst = (Wfc2.astype(np.float64) @ fb + bfc.astype(np.float64)).astype(f32)
    wpk[0:NUTT, WP_CC:WP_CC + OUT] = np.tile(cconst.reshape(1, OUT), (NUTT, 1))

    # wfc2m: [w64h fp16 | w32h fp16 | wlo fp8-bits packed as fp16]
    w64h = wfc2f[:, 48 * OUT:].astype(NPF16)
    w32h = wfc2f[:, 32 * OUT:48 * OUT].astype(NPF16)
    wlo8 = (wfc2f[:, :32 * OUT] * WLO_SCALE).astype(np8)
    wfc2m = np.concatenate(
        [w64h.view(np.uint8), w32h.view(np.uint8), wlo8.view(np.uint8)],
        axis=1).view(NPF16)

    # frames stationary: wfc1p[p, ot*256 + i*128 + o] = Wfc1[ot*100+o, 2p+i]*8
    Wfc1 = Wfc[:, :64]
    fc1p = np.zeros((32, 1024), dtype=f32)
    for ot in range(4):
        blk = Wfc1[ot * 100:(ot + 1) * 100, :]          # [100, 64]
        for i in range(2):
            fc1p[:, ot * 256 + i * 128:ot * 256 + i * 128 + 100] = \
                blk[:, i::2].T * FC1_SCALE

    return {
        "wpk16": np.ascontiguousarray(wpk.astype(NPF16)),
        "w64wp": np.ascontiguousarray(w64wp.astype(NPF16)),
        "wfc2m": np.ascontiguousarray(wfc2m),
        "fc1p": fc1p,
    }


# ---------------------------------------------------------------------------
# device program
# ---------------------------------------------------------------------------

def build_program():
    nc = bacc.Bacc("TRN2", target_bir_lowering=False, debug=False)

    dram = dict(
        xwpk=nc.dram_tensor("xwpk", [128, XW_COLS], FP16, kind="ExternalInput"),
        xtra=nc.dram_tensor("xtra", [32, XT_COLS], FP8, kind="ExternalInput"),
        w64wp=nc.dram_tensor("w64wp", [128, 8192], FP16, kind="ExternalInput"),
        wfc2m=nc.dram_tensor("wfc2m", [128, WM_COLS], FP16,
                             kind="ExternalInput"),
        out_t=nc.dram_tensor("out_t", [100, 16 * 512], FP16,
                             kind="ExternalOutput"),
    )

    with tile.TileContext(nc) as tc:
        with ExitStack() as ctx:
            _emit(nc, tc, ctx, dram)

    nc.compile()
    return nc


def _emit(nc, tc, ctx, dram):
    const = ctx.enter_context(tc.tile_pool(name="const", bufs=1))
    stg = ctx.enter_context(tc.tile_pool(name="stg", bufs=2))
    ps = ctx.enter_context(tc.tile_pool(name="ps", bufs=2, space="PSUM"))
    psc = ctx.enter_context(tc.tile_pool(name="psc", bufs=1, space="PSUM"))
    psf = ctx.enter_context(tc.tile_pool(name="psf", bufs=3, space="PSUM"))
    pstp = ctx.enter_context(tc.tile_pool(name="pstp", bufs=2, space="PSUM"))

    # ---- input tiles
    xwt = const.tile([128, XW_COLS], FP16, tag="xwt")
    xtra = const.tile([32, XT_COLS], FP8, tag="xtra")
    w64t = const.tile([128, 8192], FP16, tag="w64t")
    wfm = const.tile([128, WM_COLS], FP16, tag="wfm")

    # ---- chase loads.  Queue FIFOs (sync | scalar | gpsimd):
    #   sync:   xtra | xwpk[0:64]  | w64[0:48]   | wfc2 c3 (w32h-A) | c6 (lo-B)
    #   scalar:        xwpk[64:128]| w64[48:96]  | wfc2 c2 (h64-B)  | c5 (lo-A)
    #   gpsimd:                      w64[96:128] | wfc2 c1 (h64-A)  | c4 (w32h-B)
    # wfc2 column chunks are 3200 fp16 cols (8 kt) each; queue assignment
    # makes completion order match cmm program order: h64, h32, lo.
    sy, sc, gp = nc.sync, nc.scalar, nc.gpsimd
    sy.dma_start(xtra[:], dram["xtra"].ap())
    sy.dma_start(xwt[0:64, :], dram["xwpk"].ap()[0:64, :])
    sc.dma_start(xwt[64:128, :], dram["xwpk"].ap()[64:128, :])
    sy.dma_start(w64t[0:48, :], dram["w64wp"].ap()[0:48, :])
    sc.dma_start(w64t[48:96, :], dram["w64wp"].ap()[48:96, :])
    gp.dma_start(w64t[96:128, :], dram["w64wp"].ap()[96:128, :])
    CC = 3200
    for eng, ci in ((gp, 0), (sc, 1), (sy, 2), (gp, 3), (sc, 4), (sy, 5)):
        eng.dma_start(wfm[:, ci * CC:(ci + 1) * CC],
                      dram["wfc2m"].ap()[:, ci * CC:(ci + 1) * CC])

    xt = xwt[:, 0:W]
    w8jp = xwt[:, W + WP_W8:W + WP_W8 + 128]
    w16jp = xwt[:, W + WP_W16:W + WP_W16 + 512]
    w32jp = xwt[:, W + WP_W32:W + WP_W32 + 2048]
    cconst4 = xwt[0:NUTT, W + WP_CC:W + WP_CC + OUT]
    eye32 = xwt[0:32, W + WP_EYE:W + WP_EYE + 32]
    s8 = xwt[0:32, W + WP_S8:W + WP_S8 + 512]
    s16 = xwt[0:64, W + WP_S16:W + WP_S16 + 256]

    # ---- working tiles
    feats8 = const.tile([128, 1024], FP8, tag="feats8")
    f32t = const.tile([128, 64], FP16, tag="f32t")
    tp64sb = const.tile([128, 64], FP16, tag="tp64sb")
    ct_sb = const.tile([128, 16], FP32, tag="ct_sb")
    ct_sb16 = const.tile([128, 16], FP16, tag="ct_sb16")
    outstage = const.tile([100, 16 * 512], FP16, tag="outstage")
    warm = const.tile([128, 256], FP16, tag="warm")

    nc.vector.memset(warm[:], 0.0)
    nc.vector.memset(feats8[:], 0.0)

    # ---- PE warmup (bridges the x DMA wait; ramps the DVFS p-state)
    for _ in range(14):
        wps = psf.tile([128, 256], FP32, tag="framesps")
        nc.tensor.matmul(wps[:], warm[:, 0:128], warm[:], start=True, stop=True)

    # ---- C accumulator [32, 400]: rows 0-3 real.  Seed the whole extent
    # with one zero DR matmul so mixed DR/fp16 accumulation is defined.
    cps = psc.tile([32, OUT], FP32, tag="cps")
    zmov = feats8[:, 0:2 * OUT].rearrange("p (i n) -> p i n", i=2)
    zstat = feats8[:, 0:64].rearrange("p (i m) -> p i m", i=2)
    nc.tensor.matmul(cps[:], zstat, zmov, start=True, stop=False,
                     perf_mode=DRMODE, skip_group_check=True)

    # ---- scale h=8: 4 paired MMs K=128 M=32 N=256 -> PSUM (k*4+o, u*64+p)
    x8 = xt.rearrange("i (u p j) -> i u p j", u=NUTT, j=8)
    acc8 = ps.tile([32, NUTT * 64], FP32, tag="fsmall")
    for m in range(4):
        nc.tensor.matmul(acc8[:], w8jp[:, m * 32:(m + 1) * 32], x8[:, :, :, 2 * m],
                         start=(m == 0), stop=(m == 3))
    st8 = stg.tile([32, NUTT * 64], FP16, tag="f8st")
    nc.vector.tensor_copy(st8[:], acc8[:])

    # ---- scale h=16: 8 paired MMs K=128 M=64 N=128 -> PSUM (k*16+o, u*32+p)
    x16 = xt.rearrange("i (u p j) -> i u p j", u=NUTT, j=16)
    acc16 = ps.tile([64, NUTT * 32], FP32, tag="fsmall")
    for m in range(8):
        nc.tensor.matmul(acc16[:], w16jp[:, m * 64:(m + 1) * 64],
                         x16[:, :, :, 2 * m], start=(m == 0), stop=(m == 7))
    st16 = stg.tile([64, NUTT * 32], FP16, tag="f16st")
    nc.vector.tensor_copy(st16[:], acc16[:])

    # ---- scale h=32: 16 paired MMs K=128 M=128 N=64 -> (k*64+o, u*16+p)
    x32 = xt.rearrange("i (u p j) -> i u p j", u=NUTT, j=32)
    acc32 = ps.tile([128, NUTT * 16], FP32, tag="fsmall")
    for m in range(16):
        nc.tensor.matmul(acc32[:], w32jp[:, m * 128:(m + 1) * 128],
                         x32[:, :, :, 2 * m], start=(m == 0), stop=(m == 15))
    nc.vector.tensor_copy(f32t[:], acc32[:])  # fp = q: feeds h32 cmms directly

    # ---- partition-expansion for h8/h16: PSUM -> feats8 fp8.
    # feats8 layout: kt-pair i at cols [i*64, i*64+64): (plane, 32-padded);
    # real col(kt, u) = 32*kt + u.
    st8_r = st8[:].rearrange("q (u ph pl) -> q ph u pl", u=NUTT, ph=16)
    f8x = pstp.tile([128, 64], FP32, tag="tpps")
    for pl in range(4):
        nc.tensor.matmul(f8x[:], s8[:, pl * 128:(pl + 1) * 128],
                         st8_r[:, :, :, pl], start=(pl == 0), stop=(pl == 3))
    dst8 = feats8[:, 0:512].rearrange("p (k s) -> p k s", k=16)[:, :, 0:4]
    nc.vector.tensor_copy(dst8, f8x[:].rearrange("p (k u) -> p k u", k=16))

    st16_r = st16[:].rearrange("q (u ph pl) -> q ph u pl", u=NUTT, ph=16)
    f16x = pstp.tile([128, 64], FP32, tag="tpps")
    for pl in range(2):
        nc.tensor.matmul(f16x[:], s16[:, pl * 128:(pl + 1) * 128],
                         st16_r[:, :, :, pl], start=(pl == 0), stop=(pl == 1))
    dst16 = feats8[:, 512:1024].rearrange("p (k s) -> p k s", k=16)[:, :, 0:4]
    nc.vector.tensor_copy(dst16, f16x[:].rearrange("p (k u) -> p k u", k=16))

    # ---- frames (DR): out^T[o, t] staged to the fp16 output tile.
    # stationary wfc1p [32, 2, 128] (100 real), moving xfp8 [32, 2, 512].
    for ot in range(4):
        stat = xtra[:, XT_FC + ot * 256:XT_FC + (ot + 1) * 256].rearrange(
            "p (i m) -> p i m", i=2)
        for u in range(NUTT):
            mov = xtra[:, XT_X + u * 1024:XT_X + (u + 1) * 1024].rearrange(
                "p (i n) -> p i n", i=2)
            fps = psf.tile([128, 512], FP32, tag="framesps")
            nc.tensor.matmul(fps[:], stat, mov, start=True, stop=True,
                             perf_mode=DRMODE)
            idx = ot * NUTT + u
            dst = outstage[0:100, idx * 512:(idx + 1) * 512]
            if idx % 2:
                nc.vector.tensor_copy(dst, fps[0:100, :])
            else:
                nc.scalar.activation(dst, fps[0:100, :],
                                     mybir.ActivationFunctionType.Copy)

    # ---- scale h=64: 32 paired MMs K=128 M=32 N=256 (x stationary)
    acc64 = ps.tile([NUTT * 8, 256], FP32, tag="fsmall")
    x64 = xt.rearrange("i (u p j) -> i u p j", u=NUTT, j=64)
    for m in range(32):
        nc.tensor.matmul(acc64[:], x64[:, :, :, 2 * m],
                         w64t[:, m * 256:(m + 1) * 256],
                         start=(m == 0), stop=(m == 31))
    st64 = stg.tile([NUTT * 8, 256], FP16, tag="f64st")
    nc.vector.tensor_copy(st64[:], acc64[:])
    # PE-transpose [32, 128]x2 -> [128, 32]: tp64sb[o%128, g*32 + u*8 + p]
    for g in range(2):
        tpp = pstp.tile([128, 32], FP16, tag="tpps")
        nc.tensor.transpose(tpp[:], st64[:, g * 128:(g + 1) * 128], eye32)
        nc.vector.tensor_copy(tp64sb[:, g * 32:(g + 1) * 32], tpp[:])

    # ---- C matmuls, chasing the wfc2 column-chunk stream: h64, h32
    # (fp16 M=4), then h8/h16 as 16 fp8 DR kt-pair MMs (M=32).
    f32t_r = f32t[:, :].rearrange("q (u p) -> q p u", u=NUTT)      # [128,16,4]
    tp64_r = tp64sb[:, :].rearrange("q (g u p) -> q g p u", g=2, u=NUTT)
    for kt in range(16):
        g, p = kt // 8, kt % 8
        nc.tensor.matmul(cps[0:4, :], tp64_r[:, g, p, :],
                         wfm[:, WM_H64 + kt * OUT:WM_H64 + (kt + 1) * OUT],
                         start=False, stop=False, skip_group_check=True)
    for kt in range(16):
        nc.tensor.matmul(cps[0:4, :], f32t_r[:, kt, :],
                         wfm[:, WM_H32 + kt * OUT:WM_H32 + (kt + 1) * OUT],
                         start=False, stop=False, skip_group_check=True)
    for i in range(16):
        stat = feats8[:, i * 64:(i + 1) * 64].rearrange(
            "p (two m) -> p two m", two=2)
        mov = wfm[:, WM_LO + i * OUT:WM_LO + (i + 1) * OUT].bitcast(
            FP8).rearrange("p (two n) -> p two n", two=2)
        nc.tensor.matmul(cps[:], stat, mov, start=False, stop=(i == 15),
                         perf_mode=DRMODE, skip_group_check=True)

    # ---- C row: cps + cconst4, then PE-transpose to ct_sb[o%100, ot*4+u]
    csb16 = stg.tile([NUTT, OUT], FP16, tag="csb16")
    nc.vector.tensor_tensor(csb16[:], cps[0:4, :], cconst4, mybir.AluOpType.add)
    ctp = pstp.tile([128, 32], FP16, tag="tpps")
    for ot in range(4):
        nc.tensor.transpose(ctp[0:100, ot * 4:(ot + 1) * 4],
                            csb16[:, ot * 100:(ot + 1) * 100], eye32[0:4, 0:4])
    nc.vector.tensor_copy(ct_sb[0:100, :], ctp[0:100, 0:16])
    nc.vector.tensor_copy(ct_sb16[0:100, :], ctp[0:100, 0:16])

    # ---- finalize: outstage[p, idx*512+t] += C column (in place);
    # 6 output DMAs (3 queues x 2 column halves), each half fired as soon
    # as its adds complete.
    def ct_bcast(idx):
        base = ct_sb16[0:100, idx:idx + 1]
        return bass.AP(tensor=base.tensor, offset=base.offset,
                       ap=[[base.ap[0][0], 100], [0, 512]])

    OCH = ((0, 32), (32, 64), (64, 100))
    for half in range(2):
        for idx in range(half * 8, half * 8 + 8):
            dst = outstage[0:100, idx * 512:(idx + 1) * 512]
            e = idx % 3
            if e == 0:
                nc.scalar.activation(dst, dst,
                                     mybir.ActivationFunctionType.Identity,
                                     bias=ct_sb[0:100, idx:idx + 1], scale=1.0)
            elif e == 1:
                nc.vector.tensor_tensor(dst, dst, ct_bcast(idx),
                                        mybir.AluOpType.add)
            else:
                nc.gpsimd.tensor_tensor(dst, dst, ct_bcast(idx),
                                        mybir.AluOpType.add)
        c0, c1 = half * 8 * 512, (half + 1) * 8 * 512
        for eng, (p0, p1) in zip((sy, sc, gp), OCH):
            eng.dma_start(
                bass.AP(tensor=dram["out_t"], offset=p0 * 16 * 512 + c0,
                        ap=[[16 * 512, p1 - p0], [1, c1 - c0]]),
                outstage[p0:p1, c0:c1])


_NC_CACHE = None


def _get_nc():
    global _NC_CACHE
    if _NC_CACHE is None:
        _NC_CACHE = build_program()
    return _NC_CACHE


# ---------------------------------------------------------------------------
# entry point
# ---------------------------------------------------------------------------

def run(inputs, trace=False, **kw):
    import ml_dtypes
    np8 = ml_dtypes.float8_e4m3
    nc = _get_nc()
    prep = host_prep(inputs["W8"], inputs["b8"], inputs["W16"], inputs["b16"],
                     inputs["W32"], inputs["b32"], inputs["W64"], inputs["b64"],
                     inputs["Wfc"], inputs["bfc"])
    fc1p = prep.pop("fc1p")
    wpk16 = prep.pop("wpk16")
    batch = np.asarray(inputs["batch"], np.float32)
    in_maps = []
    for c in range(NCORES):
        x4 = batch[NUTT * c:NUTT * (c + 1)].transpose(1, 0, 2).reshape(F, W)
        x4hp = np.zeros((128, W), dtype=NPF16)
        x4hp[0:64, :] = x4.astype(NPF16)
        x4hp[64:128, 0:W - 1] = x4[:, 1:].astype(NPF16)
        # xtra: xfp8[p, u*1024 + i*512 + t] = x_u[2p+i, t] / FC1_SCALE
        xr = x4.reshape(32, 2, NUTT, T)            # [p, i, u, t]
        xtra = np.zeros((32, XT_COLS), dtype=np.float32)
        xtra[:, :XT_X + 4096] = (xr.transpose(0, 2, 1, 3) / FC1_SCALE
                                 ).reshape(32, 4096)
        xtra[:, XT_FC:] = fc1p
        m = dict(prep)
        m["xwpk"] = np.ascontiguousarray(
            np.concatenate([x4hp, wpk16], axis=1))
        m["xtra"] = np.ascontiguousarray(xtra.astype(np8))
        in_maps.append(m)
    res = run_bass_kernel_spmd(nc, in_maps, core_ids=list(range(NCORES)),
                               trace=trace, **kw)
    outs = []
    for r in res.results:
        o = r["out_t"].astype(np.float32)          # [100, 16*512]
        o = o.reshape(100, 4, NUTT, 512)           # [p, ot, u, t]
        outs.append(o.transpose(2, 3, 1, 0).reshape(W, OUT))
    return np.concatenate(outs, axis=0), res


def kernel(**inputs):
    out, _ = run(inputs)
    return out
